# revision 11
# baseline (speedup 1.0000x reference)
"""KA-attention (crossinf) Trainium2 kernel.

Math notes (exact, not approximations):
  reference computes  out = softmax_j( sum_d sigmoid(y_q)[b,h,i,d] + sum_d sigmoid(y_k)[b,h,j,d] )
  The first term is constant along the softmax axis j, so it cancels
  (softmax shift-invariance):  out[b,h,i,j] = softmax_j( B[b,h,j] ),
  B[b,h,j] = sum_d sigmoid(y_k)[b,h,j,d],  y_k = f_q * scale_sp + silu(qf) @ Wq.T.
  Only the q-path (base_weight_q, coef_q) is mathematically needed.

Sharding: tensor-shard the 8192 output rows of base_weight_q across 8 cores
(1024 rows / core).  Each core computes B for its 64 (h, j) pairs; host
gathers the 8x(16,64) partials, applies the (tiny) softmax and broadcasts
over the cancelled i axis.

Performance structure (v2):
  - Weights are quantized host-side to fp8e4 (8 MiB/core instead of 32) and
    streamed as 8 x 1 MiB chunks with 8 KiB partition rows.  The y error
    from fp8 weight+activation quantization is O(+-1) on values of O(850)
    (the matmul accumulates 8192 ~U[0,1)*silu(N(0,1)) products), far inside
    the 2e-2 output tolerance.
  - Matmul runs in DoubleRow fp8 perf mode: 256-deep contraction per
    instruction, 0.5 cycles/row -> PE stream time ~9 us (was 135 us fp32).
  - The KAN sin-spline runs in n-on-partition layout ([128, nt, b] tiles,
    all 128 partitions busy) and is transposed to the matmul's [b, n]
    layout with 8 TensorE transposes at the tail.
  - `yo` (pre-sigmoid y) is also written out; the final softmax output is
    saturation-uniform, so test harnesses should check yo to catch matmul
    or layout bugs.
"""

import sys
import numpy as np

for _p in ("/opt/trn_rl_repo", "/root/.axon_site/_ro/trn_rl_repo"):
    if _p not in sys.path:
        sys.path.append(_p)

import concourse.bass as bass
import concourse.tile as tile
from concourse import bacc, mybir
from concourse.bass_utils import run_bass_kernel_spmd
from concourse.masks import make_identity

# Problem shapes (hardcoded per contract)
B, H, P, D = 16, 4, 128, 16
NUM = H * P * D          # 8192
NF = 8                   # spline basis size
NC = 8                   # cores
NS = NUM // NC           # 1024 output rows per core
NT = NS // 128           # 8 n-tiles of 128 per core
KT = NUM // 128          # 64 k-tiles of 128
CHUNKS = [2, 2, 4, 4, 4, 4, 4, 4, 2, 2]  # DoubleRow pairs per weight DMA chunk
KT0 = 8                          # k-tiles in the first (early) silu slice
NTB = NT * B                     # 128 spline elements per partition
F32 = mybir.dt.float32
BF16 = mybir.dt.bfloat16
F8 = mybir.dt.float8e4
NP_F8 = mybir.dt.np(F8)
NP_BF16 = mybir.dt.np(BF16)

# knobs (test.py pokes these)
TRACE = False
TRACE_KW = {}
W_BUFS = 4

_CACHE = {}


def _build_bass():
    nc = bacc.Bacc("TRN2", target_bir_lowering=False, debug=False)
    # weights (kp-major): wt[kp, pair, k2, n] = Wq[n0+n, (pair*2+k2)*128 + kp]
    wt = nc.declare_dram_parameter("wt", [128, KT // 2, 2, NS], F8, isOutput=False)
    qt = nc.declare_dram_parameter("qt", [128, KT, B], BF16, isOutput=False)
    qg = nc.declare_dram_parameter("qg", [128, NTB + NF], F32, isOutput=False)
    cf = nc.declare_dram_parameter("cf", [128, NTB * NF], BF16, isOutput=False)
    bs = nc.declare_dram_parameter("bs", [B, NS // D], F32, isOutput=True)
    yo = nc.declare_dram_parameter("yo", [B, NS], F32, isOutput=True)

    with tile.TileContext(nc) as tc:
        with (
            tc.tile_pool(name="stat", bufs=1) as stat,
            tc.tile_pool(name="work", bufs=2) as work,
            tc.tile_pool(name="psum", bufs=1, space=bass.MemorySpace.PSUM) as psum,
        ):
            # static (loaded once) tiles
            qt_s = stat.tile([128, KT, B], BF16)
            sq = stat.tile([128, KT, B], F8)       # silu(qf).T, fp8 lhsT
            qg_s = stat.tile([128, NTB + NF], F32)
            cf_b = stat.tile([128, NTB * NF], BF16)
            cf_s = stat.tile([128, NTB * NF], F32)
            qs_s = qg_s[:, :NTB]
            gr_s = qg_s[:, NTB:]
            sp = stat.tile([128, NTB], F32)        # spline result (n-layout)
            ident = stat.tile([128, 128], F32)
            y = stat.tile([B, NS], F32)
            sig = stat.tile([B, NS], F32)
            bsum = stat.tile([B, NS // D], F32)

            # Each dma_start costs ~0.65 us of serial DIRECT2D descriptor
            # generation on the Sync queue; order them so the silu input and
            # the first (small) weight chunk land first, the last chunk is
            # small (its matmuls start right at the DMA drain), and the
            # spline blob rides in the middle (the spline has ~20 us slack).
            w_tiles = []
            p0 = 0
            for i, npr in enumerate(CHUNKS):
                w_i = stat.tile([128, npr, 2, NS], F8, tag=f"w{i}")
                w_tiles.append((w_i, p0, npr))
                p0 += npr
            nc.sync.dma_start(out=qt_s[:, :KT0, :], in_=qt[:, :KT0, :])
            nc.sync.dma_start(out=w_tiles[0][0],
                              in_=wt[:, 0:CHUNKS[0], :, :])
            nc.sync.dma_start(out=qg_s, in_=qg[:, :])
            nc.sync.dma_start(out=qt_s[:, KT0:, :], in_=qt[:, KT0:, :])
            nc.sync.dma_start(out=w_tiles[1][0],
                              in_=wt[:, w_tiles[1][1]:w_tiles[1][1] + CHUNKS[1], :, :])
            nc.sync.dma_start(out=cf_b, in_=cf[:, :])
            for w_i, pw, npr in w_tiles[2:]:
                nc.sync.dma_start(out=w_i, in_=wt[:, pw:pw + npr, :, :])
            make_identity(nc, ident)

            # silu(x) on ScalarE, writing the fp8 matmul operand; two halves
            # so the first starts as soon as the first half of qt lands
            nc.scalar.activation(sq[:, :KT0, :], qt_s[:, :KT0, :],
                                 mybir.ActivationFunctionType.Silu)
            nc.scalar.activation(sq[:, KT0:, :], qt_s[:, KT0:, :],
                                 mybir.ActivationFunctionType.Silu)

            # base: acc[b, n] = sum_k silu(qf)[b, k] * Wq[n0+n, k]
            # fp8 DoubleRow: each matmul contracts 256 (two k-tiles).
            acc = psum.tile([B, NS], F32)
            spT = psum.tile([B, NS], F32)
            for ci, (w_i, pw, npr) in enumerate(w_tiles):
                for pl in range(npr):
                    p = pw + pl
                    for half in range(NS // 512):
                        nc.tensor.matmul(
                            acc[:, half * 512:(half + 1) * 512],
                            sq[:, 2 * p:2 * p + 2, :],
                            w_i[:, pl, :, half * 512:(half + 1) * 512],
                            start=(p == 0),
                            stop=(p == KT // 2 - 1),
                            perf_mode=mybir.MatmulPerfMode.DoubleRow,
                        )

            # KAN sin-basis spline, all NF frequencies in one wide pass:
            # tiles are [128, NTB*NF] with f innermost, built from broadcast
            # (stride-0) views of qs and grid, so the whole spline is ~7
            # instructions instead of ~50.
            # ScalarE Sin needs args in [-pi, pi]: Cody-Waite range reduction
            # with round-to-nearest via the fp32 magic-number trick.
            INV2PI = 0.15915494309189535
            MAGIC = 12582912.0            # 1.5 * 2**23
            C1 = 6.28125                  # 2*pi split, c1 exact in fp32
            C2 = 1.9353071e-03            # fp32(2*pi - c1)
            C3 = 8.9833e-11               # remainder
            PI_CLAMP = 3.1415925          # just under fp64 pi
            mm = mybir.AluOpType
            tf = work.tile([128, NTB * NF], F32, tag="tf")
            nc.vector.tensor_tensor(
                out=tf.rearrange("p (j f) -> p j f", f=NF),
                in0=qs_s[:, :, None].broadcast_to([128, NTB, NF]),
                in1=gr_s[:, None, :].broadcast_to([128, NTB, NF]),
                op=mm.mult,
            )
            kr = work.tile([128, NTB * NF], F32, tag="kr")
            nc.vector.tensor_scalar(kr, tf, INV2PI, MAGIC,
                                    op0=mm.mult, op1=mm.add)
            k2 = work.tile([128, NTB * NF], F32, tag="k2")
            nc.vector.tensor_scalar_sub(k2, kr, MAGIC)
            red = work.tile([128, NTB * NF], F32, tag="red")
            nc.vector.cody_waite_cascade(red, tf, k2, C1, C2, C3)
            redc = work.tile([128, NTB * NF], F32, tag="redc")
            nc.vector.tensor_scalar(redc, red, PI_CLAMP, -PI_CLAMP,
                                    op0=mm.min, op1=mm.max)
            sin_t = work.tile([128, NTB * NF], F32, tag="sin")
            nc.scalar.activation(sin_t, redc,
                                 mybir.ActivationFunctionType.Sin)
            nc.vector.tensor_copy(cf_s, cf_b)
            prod = work.tile([128, NTB * NF], F32, tag="prod")
            nc.vector.tensor_mul(prod, sin_t, cf_s)
            nc.vector.reduce_sum(
                out=sp.rearrange("p (j one) -> p j one", one=1),
                in_=prod.rearrange("p (j f) -> p j f", f=NF),
                axis=mybir.AxisListType.X,
            )

            # transpose spline to the matmul layout: spT[b, nt*128+p]
            for nt_i in range(NT):
                nc.tensor.transpose(
                    spT[:, nt_i * 128:(nt_i + 1) * 128],
                    sp[:, nt_i * B:(nt_i + 1) * B], ident)

            # y = base + spline ; sig = sigmoid(y) ; B = sum over d-groups
            # (DVE has a single PSUM read port: stage spT through SBUF.)
            # Split halves so DVE add / ScalarE sigmoid / DVE reduce pipeline.
            sp_b = stat.tile([B, NS], F32)
            nc.vector.tensor_copy(sp_b, spT[:, :])
            halves = [slice(hh * 512, (hh + 1) * 512) for hh in range(2)]
            for sl in halves:
                nc.vector.tensor_add(y[:, sl], acc[:, sl], sp_b[:, sl])
            for sl in halves:
                nc.scalar.activation(sig[:, sl], y[:, sl],
                                     mybir.ActivationFunctionType.Sigmoid)
            for hh, sl in enumerate(halves):
                nc.vector.reduce_sum(
                    out=bsum[:, hh * 32:(hh + 1) * 32],
                    in_=sig[:, sl].rearrange("p (j d) -> p j d", d=D),
                    axis=mybir.AxisListType.X,
                )
            nc.sync.dma_start(out=bs[:, :], in_=bsum)
            nc.gpsimd.dma_start(out=yo[:, :], in_=y)
    nc.compile()
    return nc


def kernel(q, k, v, grid, base_weight_q, base_weight_k, coef_q, coef_k, scale_sp):
    q = np.asarray(q, dtype=np.float32)
    grid = np.asarray(grid, dtype=np.float32)
    base_weight_q = np.asarray(base_weight_q, dtype=np.float32)
    coef_q = np.asarray(coef_q, dtype=np.float32)
    scale_sp = np.asarray(scale_sp, dtype=np.float32)

    qf = q.reshape(B, NUM)
    # lhsT layout: (128, KT, B) with [kp, kt, b] = qf[b, kt*128 + kp]
    qt = np.ascontiguousarray(
        qf.T.reshape(KT, 128, B).transpose(1, 0, 2)).astype(NP_BF16)
    csc = coef_q * scale_sp[:, None]          # fold scale_sp into coef

    in_maps = []
    for c in range(NC):
        n0 = c * NS
        # wt[kp, pair, k2, n] = Wq[n0+n, (pair*2+k2)*128 + kp]
        wt = base_weight_q[n0:n0 + NS, :].T.reshape(KT // 2, 2, 128, NS)
        wt = np.ascontiguousarray(wt.transpose(2, 0, 1, 3)).astype(NP_F8)
        qg = np.zeros((128, NTB + NF), np.float32)
        # qs[p, nt*B+b] = qf[b, n0 + nt*128 + p]
        qg[:, :NTB] = (
            qf[:, n0:n0 + NS].T.reshape(NT, 128, B).transpose(1, 0, 2)
        ).reshape(128, NTB)
        qg[:, NTB:] = grid[None, :]
        # cf[p, ((nt, b), f)] = csc[n0 + nt*128 + p, f]  (broadcast over b)
        cfc = csc[n0:n0 + NS, :].reshape(NT, 128, NF).transpose(1, 0, 2)
        cfb = np.broadcast_to(cfc[:, :, None, :], (128, NT, B, NF))
        in_maps.append({"wt": wt, "qt": qt, "qg": qg,
                        "cf": np.ascontiguousarray(cfb).reshape(
                            128, NTB * NF).astype(NP_BF16)})

    if "nc" not in _CACHE:
        _CACHE["nc"] = _build_bass()
    res = run_bass_kernel_spmd(_CACHE["nc"], in_maps, list(range(NC)),
                               trace=TRACE, **TRACE_KW)
    _CACHE["last_result"] = res

    Bmat = np.empty((B, H, P), np.float32)
    for c in range(NC):
        h, j0 = c // 2, 64 * (c % 2)
        Bmat[:, h, j0:j0 + 64] = res.results[c]["bs"]

    # softmax over j (float32, same stabilized form jax uses)
    m = Bmat.max(axis=-1, keepdims=True)
    e = np.exp(Bmat - m)
    soft = (e / e.sum(axis=-1, keepdims=True)).astype(np.float32)
    return np.ascontiguousarray(
        np.broadcast_to(soft[:, :, None, :], (B, H, P, P)))


# revision 12
# speedup vs baseline: 1.0951x; 1.0951x over previous
"""KA-attention (crossinf) Trainium2 kernel.

Math notes (exact, not approximations):
  reference computes  out = softmax_j( sum_d sigmoid(y_q)[b,h,i,d] + sum_d sigmoid(y_k)[b,h,j,d] )
  The first term is constant along the softmax axis j, so it cancels
  (softmax shift-invariance):  out[b,h,i,j] = softmax_j( B[b,h,j] ),
  B[b,h,j] = sum_d sigmoid(y_k)[b,h,j,d],  y_k = f_q * scale_sp + silu(qf) @ Wq.T.
  Only the q-path (base_weight_q, coef_q) is mathematically needed.

Sharding: tensor-shard the 8192 output rows of base_weight_q across 8 cores
(1024 rows / core).  Each core computes B for its 64 (h, j) pairs; host
gathers the 8x(16,64) partials, applies the (tiny) softmax and broadcasts
over the cancelled i axis.

Performance structure (v2):
  - Weights are quantized host-side to fp8e4 (8 MiB/core instead of 32) and
    streamed as 8 x 1 MiB chunks with 8 KiB partition rows.  The y error
    from fp8 weight+activation quantization is O(+-1) on values of O(850)
    (the matmul accumulates 8192 ~U[0,1)*silu(N(0,1)) products), far inside
    the 2e-2 output tolerance.
  - Matmul runs in DoubleRow fp8 perf mode: 256-deep contraction per
    instruction, 0.5 cycles/row -> PE stream time ~9 us (was 135 us fp32).
  - The KAN sin-spline runs in n-on-partition layout ([128, nt, b] tiles,
    all 128 partitions busy) and is transposed to the matmul's [b, n]
    layout with 8 TensorE transposes at the tail.
  - `yo` (pre-sigmoid y) is also written out; the final softmax output is
    saturation-uniform, so test harnesses should check yo to catch matmul
    or layout bugs.
"""

import sys
import numpy as np

for _p in ("/opt/trn_rl_repo", "/root/.axon_site/_ro/trn_rl_repo"):
    if _p not in sys.path:
        sys.path.append(_p)

import concourse.bass as bass
import concourse.tile as tile
from concourse import bacc, mybir
from concourse.bass_utils import run_bass_kernel_spmd
from concourse.masks import make_identity

# Problem shapes (hardcoded per contract)
B, H, P, D = 16, 4, 128, 16
NUM = H * P * D          # 8192
NF = 8                   # spline basis size
NC = 8                   # cores
NS = NUM // NC           # 1024 output rows per core
NT = NS // 128           # 8 n-tiles of 128 per core
KT = NUM // 128          # 64 k-tiles of 128
CHUNKS = [2, 2, 4, 4, 4, 4, 4, 4, 2, 2]  # DoubleRow pairs per weight DMA chunk
KT0 = 8                          # k-tiles in the first (early) silu slice
NTB = NT * B                     # 128 spline elements per partition
F32 = mybir.dt.float32
BF16 = mybir.dt.bfloat16
F8 = mybir.dt.float8e4
NP_F8 = mybir.dt.np(F8)
NP_BF16 = mybir.dt.np(BF16)

# knobs (test.py pokes these)
TRACE = False
TRACE_KW = {}
W_BUFS = 4

_CACHE = {}


def _build_bass():
    nc = bacc.Bacc("TRN2", target_bir_lowering=False, debug=False)
    # weights (kp-major): wt[kp, pair, k2, n] = Wq[n0+n, (pair*2+k2)*128 + kp]
    wt = nc.declare_dram_parameter("wt", [128, KT // 2, 2, NS], F8, isOutput=False)
    qt = nc.declare_dram_parameter("qt", [128, KT, B], BF16, isOutput=False)
    qg = nc.declare_dram_parameter("qg", [128, NTB + NF], F32, isOutput=False)
    cf = nc.declare_dram_parameter("cf", [128, NTB * NF], BF16, isOutput=False)
    bs = nc.declare_dram_parameter("bs", [B, NS // D], F32, isOutput=True)
    yo = nc.declare_dram_parameter("yo", [B, NS], F32, isOutput=True)

    with tile.TileContext(nc) as tc:
        with (
            tc.tile_pool(name="stat", bufs=1) as stat,
            tc.tile_pool(name="work", bufs=2) as work,
            tc.tile_pool(name="psum", bufs=1, space=bass.MemorySpace.PSUM) as psum,
        ):
            # static (loaded once) tiles
            qt_s = stat.tile([128, KT, B], BF16)
            sq = stat.tile([128, KT, B], F8)       # silu(qf).T, fp8 lhsT
            qg_s = stat.tile([128, NTB + NF], F32)
            cf_b = stat.tile([128, NTB * NF], BF16)
            cf_s = stat.tile([128, NTB * NF], F32)
            qs_s = qg_s[:, :NTB]
            gr_s = qg_s[:, NTB:]
            sp = stat.tile([128, NTB], F32)        # spline result (n-layout)
            ident = stat.tile([128, 128], F32)
            y = stat.tile([B, NS], F32)
            sig = stat.tile([B, NS], F32)
            bsum = stat.tile([B, NS // D], F32)

            # Each dma_start costs ~0.65 us of serial DIRECT2D descriptor
            # generation on the Sync queue; order them so the silu input and
            # the first (small) weight chunk land first, the last chunk is
            # small (its matmuls start right at the DMA drain), and the
            # spline blob rides in the middle (the spline has ~20 us slack).
            w_tiles = []
            p0 = 0
            for i, npr in enumerate(CHUNKS):
                w_i = stat.tile([128, npr, 2, NS], F8, tag=f"w{i}")
                w_tiles.append((w_i, p0, npr))
                p0 += npr
            nc.sync.dma_start(out=qt_s[:, :KT0, :], in_=qt[:, :KT0, :])
            nc.sync.dma_start(out=w_tiles[0][0],
                              in_=wt[:, 0:CHUNKS[0], :, :])
            nc.sync.dma_start(out=qg_s, in_=qg[:, :])
            nc.sync.dma_start(out=qt_s[:, KT0:, :], in_=qt[:, KT0:, :])
            nc.sync.dma_start(out=w_tiles[1][0],
                              in_=wt[:, w_tiles[1][1]:w_tiles[1][1] + CHUNKS[1], :, :])
            nc.sync.dma_start(out=cf_b, in_=cf[:, :])
            for w_i, pw, npr in w_tiles[2:]:
                nc.sync.dma_start(out=w_i, in_=wt[:, pw:pw + npr, :, :])
            make_identity(nc, ident)

            # silu(x) on ScalarE, writing the fp8 matmul operand; two halves
            # so the first starts as soon as the first half of qt lands
            nc.scalar.activation(sq[:, :KT0, :], qt_s[:, :KT0, :],
                                 mybir.ActivationFunctionType.Silu)
            nc.scalar.activation(sq[:, KT0:, :], qt_s[:, KT0:, :],
                                 mybir.ActivationFunctionType.Silu)

            # base: acc[b, n] = sum_k silu(qf)[b, k] * Wq[n0+n, k]
            # fp8 DoubleRow: each matmul contracts 256 (two k-tiles).
            acc = psum.tile([B, NS], F32)
            spT = psum.tile([B, NS], F32)
            for ci, (w_i, pw, npr) in enumerate(w_tiles):
                for pl in range(npr):
                    p = pw + pl
                    for half in range(NS // 512):
                        nc.tensor.matmul(
                            acc[:, half * 512:(half + 1) * 512],
                            sq[:, 2 * p:2 * p + 2, :],
                            w_i[:, pl, :, half * 512:(half + 1) * 512],
                            start=(p == 0),
                            stop=(p == KT // 2 - 1),
                            perf_mode=mybir.MatmulPerfMode.DoubleRow,
                        )

            # KAN sin-basis spline, all NF frequencies in one wide pass:
            # tiles are [128, NTB*NF] with f innermost, built from broadcast
            # (stride-0) views of qs and grid, so the whole spline is ~7
            # instructions instead of ~50.
            # ScalarE Sin needs args in [-pi, pi]: Cody-Waite range reduction
            # with round-to-nearest via the fp32 magic-number trick.
            INV2PI = 0.15915494309189535
            MAGIC = 12582912.0            # 1.5 * 2**23
            C1 = 6.28125                  # 2*pi split, c1 exact in fp32
            C2 = 1.9353071e-03            # fp32(2*pi - c1)
            C3 = 8.9833e-11               # remainder
            PI_CLAMP = 3.1415925          # just under fp64 pi
            mm = mybir.AluOpType
            tf = work.tile([128, NTB * NF], F32, tag="tf")
            nc.vector.tensor_tensor(
                out=tf.rearrange("p (j f) -> p j f", f=NF),
                in0=qs_s[:, :, None].broadcast_to([128, NTB, NF]),
                in1=gr_s[:, None, :].broadcast_to([128, NTB, NF]),
                op=mm.mult,
            )
            kr = work.tile([128, NTB * NF], F32, tag="kr")
            nc.vector.tensor_scalar(kr, tf, INV2PI, MAGIC,
                                    op0=mm.mult, op1=mm.add)
            k2 = work.tile([128, NTB * NF], F32, tag="k2")
            nc.vector.tensor_scalar_sub(k2, kr, MAGIC)
            red = work.tile([128, NTB * NF], F32, tag="red")
            nc.vector.cody_waite_cascade(red, tf, k2, C1, C2, C3)
            redc = work.tile([128, NTB * NF], F32, tag="redc")
            nc.vector.tensor_scalar(redc, red, PI_CLAMP, -PI_CLAMP,
                                    op0=mm.min, op1=mm.max)
            sin_t = work.tile([128, NTB * NF], F32, tag="sin")
            nc.scalar.activation(sin_t, redc,
                                 mybir.ActivationFunctionType.Sin)
            nc.vector.tensor_copy(cf_s, cf_b)
            prod = work.tile([128, NTB * NF], F32, tag="prod")
            nc.vector.tensor_mul(prod, sin_t, cf_s)
            nc.vector.reduce_sum(
                out=sp.rearrange("p (j one) -> p j one", one=1),
                in_=prod.rearrange("p (j f) -> p j f", f=NF),
                axis=mybir.AxisListType.X,
            )

            # transpose spline to the matmul layout: spT[b, nt*128+p]
            for nt_i in range(NT):
                nc.tensor.transpose(
                    spT[:, nt_i * 128:(nt_i + 1) * 128],
                    sp[:, nt_i * B:(nt_i + 1) * B], ident)

            # y = base + spline ; sig = sigmoid(y) ; B = sum over d-groups
            # (DVE has a single PSUM read port: stage spT through SBUF.)
            # Split halves so DVE add / ScalarE sigmoid / DVE reduce pipeline.
            sp_b = stat.tile([B, NS], F32)
            nc.vector.tensor_copy(sp_b, spT[:, :])
            halves = [slice(hh * 512, (hh + 1) * 512) for hh in range(2)]
            for sl in halves:
                nc.vector.tensor_add(y[:, sl], acc[:, sl], sp_b[:, sl])
            for sl in halves:
                nc.scalar.activation(sig[:, sl], y[:, sl],
                                     mybir.ActivationFunctionType.Sigmoid)
            for hh, sl in enumerate(halves):
                nc.vector.reduce_sum(
                    out=bsum[:, hh * 32:(hh + 1) * 32],
                    in_=sig[:, sl].rearrange("p (j d) -> p j d", d=D),
                    axis=mybir.AxisListType.X,
                )
            nc.sync.dma_start(out=bs[:, :], in_=bsum)
            nc.sync.dma_start(out=yo[:, :], in_=y)
    nc.compile()
    return nc


def kernel(q, k, v, grid, base_weight_q, base_weight_k, coef_q, coef_k, scale_sp):
    q = np.asarray(q, dtype=np.float32)
    grid = np.asarray(grid, dtype=np.float32)
    base_weight_q = np.asarray(base_weight_q, dtype=np.float32)
    coef_q = np.asarray(coef_q, dtype=np.float32)
    scale_sp = np.asarray(scale_sp, dtype=np.float32)

    qf = q.reshape(B, NUM)
    # lhsT layout: (128, KT, B) with [kp, kt, b] = qf[b, kt*128 + kp]
    qt = np.ascontiguousarray(
        qf.T.reshape(KT, 128, B).transpose(1, 0, 2)).astype(NP_BF16)
    csc = coef_q * scale_sp[:, None]          # fold scale_sp into coef

    in_maps = []
    for c in range(NC):
        n0 = c * NS
        # wt[kp, pair, k2, n] = Wq[n0+n, (pair*2+k2)*128 + kp]
        wt = base_weight_q[n0:n0 + NS, :].T.reshape(KT // 2, 2, 128, NS)
        wt = np.ascontiguousarray(wt.transpose(2, 0, 1, 3)).astype(NP_F8)
        qg = np.zeros((128, NTB + NF), np.float32)
        # qs[p, nt*B+b] = qf[b, n0 + nt*128 + p]
        qg[:, :NTB] = (
            qf[:, n0:n0 + NS].T.reshape(NT, 128, B).transpose(1, 0, 2)
        ).reshape(128, NTB)
        qg[:, NTB:] = grid[None, :]
        # cf[p, ((nt, b), f)] = csc[n0 + nt*128 + p, f]  (broadcast over b)
        cfc = csc[n0:n0 + NS, :].reshape(NT, 128, NF).transpose(1, 0, 2)
        cfb = np.broadcast_to(cfc[:, :, None, :], (128, NT, B, NF))
        in_maps.append({"wt": wt, "qt": qt, "qg": qg,
                        "cf": np.ascontiguousarray(cfb).reshape(
                            128, NTB * NF).astype(NP_BF16)})

    if "nc" not in _CACHE:
        _CACHE["nc"] = _build_bass()
    res = run_bass_kernel_spmd(_CACHE["nc"], in_maps, list(range(NC)),
                               trace=TRACE, **TRACE_KW)
    _CACHE["last_result"] = res

    Bmat = np.empty((B, H, P), np.float32)
    for c in range(NC):
        h, j0 = c // 2, 64 * (c % 2)
        Bmat[:, h, j0:j0 + 64] = res.results[c]["bs"]

    # softmax over j (float32, same stabilized form jax uses)
    m = Bmat.max(axis=-1, keepdims=True)
    e = np.exp(Bmat - m)
    soft = (e / e.sum(axis=-1, keepdims=True)).astype(np.float32)
    return np.ascontiguousarray(
        np.broadcast_to(soft[:, :, None, :], (B, H, P, P)))
